# revision 24
# baseline (speedup 1.0000x reference)
"""Trainium2 Bass kernel for e3nn-style GNN message passing + segment-sum.

Strategy (v2 — batched bf16 pipeline, dma_gather):
  - Sharding: core c owns nodes [c*N/8, (c+1)*N/8) and receives exactly the
    edges targeting them (edge shard = mask-compaction of edge_feats rows in
    natural edge order; bf16).  No collectives: output rows are exclusive.
  - Host (numpy, index prep): windows of S=128 nodes; edges of a window are
    packed into 128-edge chunks.  Each core's edge table is split into two
    <=32768-row halves so dma_gather's int16 indices reach every row; chunks
    are half-homogeneous.  Host pre-packs per slot: gather index (wrapped-16
    int16 layout), edge attrs (s0,s0,v0x,v0y,v0z,0,0,0 — s0 duplicated so
    broadcast APs keep the DVE 2x perf mode), rel-receiver (f32, for the
    one-hot), the MLP scalars pre-transposed ([64, 512] per 8-chunk batch for
    block-diagonal matmuls), and the scalars tripled (s[c] at cols 3c+d).
  - Device (Bass/Tile, SPMD on 8 cores): per window
      * one dma_gather per table-half pulls the window's edge rows (256B/row)
      * MLP 8 chunks at a time: block-diag W1/W2 keep 128 partitions busy;
        W3 pre-expanded to 256 cols (vector gates replicated x3), one ACT
        copy yields all gates edge-major bf16
      * tensor-product messages on DVE, batched, bf16 (2x mode layouts)
      * per chunk: one-hot(rel) via tensor_scalar is_equal, then ONE
        accumulating matmul win[128 nodes, 256 feats] += oh^T @ msg
      * window flush: ACT copy PSUM->SBUF, DMA out in [node, feat] layout.
"""

import math
import os

import numpy as np
import ml_dtypes

BF16 = ml_dtypes.bfloat16

P = 128          # edges per chunk == SBUF partitions
S = 128          # nodes per window (one-hot width, PSUM partition dim)
MB = 8           # chunks per MLP/DVE batch
TS = 32768       # rows per gather table half (int16 index range)
C32 = 32         # irrep multiplicity
NCORES = 8

_CACHE = {}
_LAST_RUN = [None, None]


def last_run():
    """(nc, in_maps) of the most recent kernel() call — for external timing."""
    return _LAST_RUN[0], _LAST_RUN[1]


# ----------------------------------------------------------------- host prep

def _build_schedule(rc, n, ncores):
    """Per-core slot tables.

    Returns (K_wt [nw,2] shared chunk counts, per_core) with per_core[c] =
    (eids [cnt] global edge ids of the core's table rows (natural order),
     loc_km [C,P] int64 local table row per slot (dummy 0 on pads),
     ids_km [C,P] int64 global edge id per slot (0 on pads),
     rel_km [C,P] int64 rel receiver (-1 on pads)).
    """
    npc = n // ncores
    assert n % ncores == 0
    nw = math.ceil(npc / S)

    cores = []
    for c in range(ncores):
        eids = np.flatnonzero(rc // npc == c)          # ascending edge ids
        cnt = eids.shape[0]
        assert cnt <= 2 * TS, f"core {c} edge count {cnt} exceeds 2*{TS}"
        h = min(cnt, TS)
        loc = np.arange(cnt, dtype=np.int64)
        half = (loc >= h).astype(np.int64)
        w = (rc[eids] - c * npc) // S
        cores.append((eids, loc, half, w, h))

    K_wt = np.zeros((nw, 2), dtype=np.int64)
    for c in range(ncores):
        _, _, half, w, _ = cores[c]
        for t in (0, 1):
            cnts = np.bincount(w[half == t], minlength=nw)
            K_wt[:, t] = np.maximum(K_wt[:, t], -(-cnts // P))
    # every window needs >= 1 chunk so the PSUM accumulation group exists
    empty = (K_wt.sum(axis=1) == 0)
    K_wt[empty, 0] = 1

    C = int(K_wt.sum())
    per_core = []
    for c in range(ncores):
        eids, loc, half, w, h = cores[c]
        loc_km = np.zeros((C, P), np.int64)
        ids_km = np.zeros((C, P), np.int64)
        rel_km = np.full((C, P), -1, np.int64)
        c0 = 0
        for wi in range(nw):
            for t in (0, 1):
                K = int(K_wt[wi, t])
                if K == 0:
                    continue
                sel = np.flatnonzero((w == wi) & (half == t))
                nreal = sel.shape[0]
                tot = K * P
                l = np.zeros(tot, np.int64)
                g = np.zeros(tot, np.int64)
                r = np.full(tot, -1, np.int64)
                l[:nreal] = loc[sel] - t * h
                g[:nreal] = eids[sel]
                r[:nreal] = rc[eids[sel]] - (c * npc + wi * S)
                loc_km[c0:c0 + K] = l.reshape(K, P)
                ids_km[c0:c0 + K] = g.reshape(K, P)
                rel_km[c0:c0 + K] = r.reshape(K, P)
                c0 += K
        per_core.append((eids, loc_km, ids_km, rel_km))
    return K_wt, per_core


def _batches(K_wt):
    """Static batch list: (window, chunk_base_in_window, n_chunks)."""
    nw = K_wt.shape[0]
    out = []
    base = 0
    for w in range(nw):
        K = int(K_wt[w].sum())
        for b0 in range(0, K, MB):
            out.append((w, base, b0, min(MB, K - b0)))
        base += K
    return out


def _pack_idx16(loc_km, K_wt):
    """Wrapped-16 int16 index stream: per (w,t) group, idx i of the group at
    [i%16, i//16], replicated over the 8 partition groups -> [128, 8*C]."""
    C = loc_km.shape[0]
    flat = loc_km.reshape(C * P).astype(np.int16)        # group-major already
    wrapped = flat.reshape(C * 8, 16).T                  # [16, 8*C]
    return np.ascontiguousarray(np.tile(wrapped, (8, 1)))


def _pack_core(ef, ea, eids, loc_km, ids_km, rel_km, K_wt, batches):
    Cn = ids_km.shape[0]
    valid = rel_km >= 0

    tab = np.zeros((2 * TS, ef.shape[1]), np.float32)
    tab[:eids.shape[0]] = ef[eids]
    tab16 = tab.astype(BF16)

    idx16 = _pack_idx16(loc_km, K_wt)

    at = np.zeros((Cn, P, 8), np.float32)
    ea4 = ea[ids_km]
    at[:, :, 0] = ea4[:, :, 0]
    at[:, :, 1] = ea4[:, :, 0]
    at[:, :, 2:5] = ea4[:, :, 1:4]
    at[~valid] = 0.0
    attrs = np.ascontiguousarray(
        at.transpose(1, 0, 2).reshape(P, Cn * 8)).astype(BF16)

    relpm = np.ascontiguousarray(rel_km.T.astype(np.float32))

    s = ef[ids_km, :C32]                                  # [C,P,32]
    s[~valid] = 0.0
    st3 = np.repeat(s, 3, axis=2)
    st3p = np.ascontiguousarray(
        st3.transpose(1, 0, 2).reshape(P, Cn * 96)).astype(BF16)

    nbt = len(batches)
    sb8 = np.zeros((nbt, 8, P, C32), np.float32)
    for ib, (w, base, b0, nch) in enumerate(batches):
        g0 = base + b0
        sb8[ib, :nch] = s[g0:g0 + nch]
    sre = sb8.reshape(nbt, 2, 4, P, C32).transpose(0, 1, 4, 2, 3)
    sT = np.ascontiguousarray(
        sre.reshape(nbt, 64, 4 * P).transpose(1, 0, 2).reshape(64, nbt * 4 * P)
    ).astype(BF16)
    return tab16, idx16, attrs, relpm, st3p, sT


# ------------------------------------------------------------- device build

def _build_program(K_wt, batches, ncores):
    import concourse.bass as bass
    import concourse.bacc as bacc
    import concourse.mybir as mybir
    import concourse.tile as tile
    from concourse import library_config

    F32 = mybir.dt.float32
    BF = mybir.dt.bfloat16
    I16 = mybir.dt.int16
    AF = mybir.ActivationFunctionType
    ALU = mybir.AluOpType
    nw = K_wt.shape[0]
    Cn = int(K_wt.sum())
    nbt = len(batches)
    kstage = int(os.environ.get("KSTAGE", "6"))

    nc = bacc.Bacc("TRN2", target_bir_lowering=False, debug=False,
                   num_devices=ncores)

    ef_d = nc.dram_tensor("ef", [2 * TS, P], BF, kind="ExternalInput")
    idx_d = nc.dram_tensor("idx", [P, 8 * Cn], I16, kind="ExternalInput")
    rel_d = nc.dram_tensor("rel", [P, Cn], F32, kind="ExternalInput")
    attrs_d = nc.dram_tensor("attrs", [P, 8 * Cn], BF, kind="ExternalInput")
    st3_d = nc.dram_tensor("st3", [P, 96 * Cn], BF, kind="ExternalInput")
    sT_d = nc.dram_tensor("sT", [64, 4 * P * nbt], BF, kind="ExternalInput")
    iota_d = nc.dram_tensor("iota", [P, S], BF, kind="ExternalInput")
    w1_d = nc.dram_tensor("w1", [64, 128], BF, kind="ExternalInput")
    w2_d = nc.dram_tensor("w2", [128, 128], BF, kind="ExternalInput")
    w3_d = nc.dram_tensor("w3", [128, 256], BF, kind="ExternalInput")
    out_d = nc.dram_tensor("out", [nw * S, 2 * P], F32, kind="ExternalOutput")

    with tile.TileContext(nc) as tc:
        with tc.tile_pool(name="const", bufs=1) as cpool, \
             tc.tile_pool(name="gef", bufs=2) as gef, \
             tc.tile_pool(name="sbat", bufs=3) as sbat, \
             tc.tile_pool(name="smsg", bufs=2) as smsg, \
             tc.tile_pool(name="soh", bufs=16) as soh, \
             tc.tile_pool(name="wfp", bufs=2) as wfp, \
             tc.tile_pool(name="pmm", bufs=1, space="PSUM") as pmm, \
             tc.tile_pool(name="pwin", bufs=2, space="PSUM") as pwin:

            nc.gpsimd.load_library(library_config.mlp)
            iota = cpool.tile([P, S], BF)
            w1b = cpool.tile([64, 128], BF)
            w2b = cpool.tile([128, 128], BF)
            w3b = cpool.tile([128, 256], BF)
            idx_sb = cpool.tile([P, 8 * Cn], I16)
            rel_sb = cpool.tile([P, Cn], F32)
            nc.sync.dma_start(iota[:], iota_d[:])
            nc.sync.dma_start(w1b[:], w1_d[:])
            nc.sync.dma_start(w2b[:], w2_d[:])
            nc.sync.dma_start(w3b[:], w3_d[:])
            nc.sync.dma_start(idx_sb[:], idx_d[:])
            nc.sync.dma_start(rel_sb[:], rel_d[:])

            # per-window bookkeeping
            wbase = np.concatenate([[0], np.cumsum(K_wt.sum(axis=1))])
            efw_tiles = {}
            win_tiles = {}

            for ib, (w, base, b0, nch) in enumerate(batches):
                if b0 == 0:
                    # window start: gather both table halves
                    Kw = int(K_wt[w].sum())
                    Kmax = int(K_wt.sum(axis=1).max())
                    efw = gef.tile([P, Kmax * P], BF, tag="efw", name="efw")
                    efw_tiles[w] = efw
                    coff = 0
                    for t in (0, 1):
                        K = int(K_wt[w, t])
                        if K == 0:
                            continue
                        ni = K * P
                        g0 = base + coff
                        nc.gpsimd.dma_gather(
                            efw[:, coff * P:(coff + K) * P].rearrange(
                                "p (j f) -> p j f", f=P),
                            ef_d[t * TS:(t + 1) * TS, :],
                            idx_sb[:, 8 * g0:8 * (g0 + K)],
                            ni, ni, P, single_packet=False)
                        coff += K
                    win_tiles[w] = pwin.tile([S, 2 * P], F32, tag="win",
                                             space="PSUM", name="win")
                efw = efw_tiles[w]
                win = win_tiles[w]
                gch = base + b0            # global chunk id of batch start
                Kw = int(K_wt[w].sum())

                if kstage < 2:
                    continue
                sTb = sbat.tile([64, 4 * P], BF, tag="sT")
                st3b = sbat.tile([P, MB * 96], BF, tag="st3")
                atb = sbat.tile([P, MB * 8], BF, tag="at")
                nc.sync.dma_start(sTb[:], sT_d[:, ib * 4 * P:(ib + 1) * 4 * P])
                nc.sync.dma_start(
                    st3b[:, 0:nch * 96],
                    st3_d[:, gch * 96:(gch + nch) * 96])
                nc.sync.dma_start(
                    atb[:, 0:nch * 8], attrs_d[:, gch * 8:(gch + nch) * 8])

                if kstage < 3:
                    continue
                # ---- MLP: h1/h2 partition-stacked via block-diag weights
                h1 = pmm.tile([P, 4 * P], F32, tag="h1", space="PSUM")
                nc.tensor.matmul(h1[:], w1b[:], sTb[:], start=True, stop=True)
                h1s = smsg.tile([P, 4 * P], BF, tag="h1s")
                nc.scalar.activation(h1s[:], h1[:], AF.Silu)
                h2 = pmm.tile([P, 4 * P], F32, tag="h2", space="PSUM")
                nc.tensor.matmul(h2[:], w2b[:], h1s[:], start=True, stop=True)
                h2s = smsg.tile([P, 4 * P], BF, tag="h2s")
                nc.scalar.activation(h2s[:], h2[:], AF.Silu)
                mixx = pmm.tile([P, MB * 256], F32, tag="mixx", space="PSUM")
                for k in range(nch):
                    half = (k // 4) * 64
                    lhsT = h2s[half:half + 64, (k % 4) * P:(k % 4) * P + P]
                    nc.tensor.matmul(mixx[:, k * 256:(k + 1) * 256],
                                     lhsT, w3b[half:half + 64, :],
                                     start=True, stop=True)
                mixxs = smsg.tile([P, MB * 256], BF, tag="mixxs")
                nc.scalar.activation(mixxs[:, 0:nch * 256],
                                     mixx[:, 0:nch * 256], AF.Copy)

                if kstage < 4:
                    continue
                # ---- messages (DVE, batched over nch chunks, bf16)
                efb = efw[:, b0 * P:(b0 + nch) * P]
                ef3 = efb.rearrange("p (j f) -> p j f", f=P)
                at3 = atb[:, 0:nch * 8].rearrange("p (j a) -> p j a", a=8)
                mx3 = mixxs[:, 0:nch * 256].rearrange("p (j f) -> p j f",
                                                      f=256)
                v4 = ef3[:, :, C32:4 * C32].rearrange(
                    "p j (c d) -> p j c d", d=3)
                v0b = at3[:, :, 2:5].rearrange(
                    "p j (x d) -> p j x d", x=1).to_broadcast(
                        [P, nch, C32, 3])
                s0p = at3[:, :, 0:2].rearrange("p j (x d) -> p j x d", x=1)

                t1 = smsg.tile([P, MB * 96], BF, tag="t1")
                t14 = t1[:, 0:nch * 96].rearrange(
                    "p (j c d) -> p j c d", c=C32, d=3)
                nc.vector.tensor_tensor(t14, v4, v0b, op=ALU.mult)

                ab = smsg.tile([P, MB * 64], BF, tag="ab")
                ab3 = ab[:, 0:nch * 64].rearrange("p (j f) -> p j f", f=64)
                with nc.allow_low_precision("bf16 edge dot accum"):
                    nc.vector.tensor_reduce(
                        ab3[:, :, C32:2 * C32], t14,
                        axis=mybir.AxisListType.X, op=ALU.add)
                a4 = ab3[:, :, 0:C32].rearrange("p j (c d) -> p j c d", d=2)
                sin4 = ef3[:, :, 0:C32].rearrange("p j (c d) -> p j c d", d=2)
                nc.vector.tensor_tensor(
                    a4, sin4, s0p.to_broadcast([P, nch, 16, 2]), op=ALU.mult)

                msg = smsg.tile([P, MB * 256], BF, tag="msg")
                m3 = msg[:, 0:nch * 256].rearrange("p (j f) -> p j f", f=256)
                nc.vector.tensor_tensor(m3[:, :, 0:64], ab3[:],
                                        mx3[:, :, 0:64], op=ALU.mult)

                st34 = st3b[:, 0:nch * 96].rearrange(
                    "p (j c d) -> p j c d", c=C32, d=3)
                tv = smsg.tile([P, MB * 96], BF, tag="tv")
                tv3 = tv[:, 0:nch * 96].rearrange("p (j f) -> p j f", f=96)
                nc.vector.tensor_tensor(
                    tv3.rearrange("p j (c d) -> p j c d", c=C32, d=3),
                    st34, v0b, op=ALU.mult)
                nc.vector.tensor_tensor(m3[:, :, 64:160], tv3,
                                        mx3[:, :, 64:160], op=ALU.mult)

                vv = smsg.tile([P, MB * 96], BF, tag="vv")
                vv3 = vv[:, 0:nch * 96].rearrange("p (j f) -> p j f", f=96)
                vin4 = ef3[:, :, C32:4 * C32].rearrange(
                    "p j (c d) -> p j c d", d=2)
                nc.vector.tensor_tensor(
                    vv3.rearrange("p j (c d) -> p j c d", c=48, d=2),
                    vin4, s0p.to_broadcast([P, nch, 48, 2]), op=ALU.mult)
                nc.vector.tensor_tensor(m3[:, :, 160:256], vv3,
                                        mx3[:, :, 160:256], op=ALU.mult)

                if kstage < 5:
                    continue
                # ---- one-hot (one batched op) + window accumulation
                oh = soh.tile([P, MB * S], BF, tag="oh")
                nc.vector.tensor_tensor(
                    oh[:, 0:nch * S].rearrange("p (j f) -> p j f", f=S),
                    iota[:].rearrange("p (x f) -> p x f", x=1)
                           .to_broadcast([P, nch, S]),
                    rel_sb[:, gch:gch + nch].rearrange(
                        "p (j x) -> p j x", x=1).to_broadcast([P, nch, S]),
                    op=ALU.is_equal)
                for k in range(nch):
                    nc.tensor.matmul(win[:], oh[:, k * S:(k + 1) * S],
                                     msg[:, k * 256:(k + 1) * 256],
                                     start=(b0 + k == 0),
                                     stop=(b0 + k == Kw - 1))
                if b0 + nch == Kw:
                    wf = wfp.tile([S, 2 * P], F32, tag="wf")
                    nc.scalar.activation(wf[:], win[:], AF.Copy)
                    nc.sync.dma_start(out_d[w * S:(w + 1) * S, :], wf[:])
                    del efw_tiles[w], win_tiles[w]

    nc.compile()
    return nc


# ------------------------------------------------------------------- kernel

def kernel(edge_feats, edge_attrs, receivers, n_nodes, W1, W2, W3):
    from concourse.bass_utils import run_bass_kernel_spmd

    ef = np.asarray(edge_feats, dtype=np.float32)
    ea = np.asarray(edge_attrs, dtype=np.float32)
    rc = np.asarray(receivers).astype(np.int64)
    n = int(n_nodes)
    W1 = np.asarray(W1, dtype=np.float32)
    W2 = np.asarray(W2, dtype=np.float32)
    W3 = np.asarray(W3, dtype=np.float32)
    npc = n // NCORES
    nw = math.ceil(npc / S)

    K_wt, per_core = _build_schedule(rc, n, NCORES)
    batches = _batches(K_wt)

    # prescaled weights; fold 1/sqrt(3) (cols 32:64) and 1/sqrt(20) into W3
    w1s = (W1 / math.sqrt(W1.shape[0])).astype(np.float32)
    w2s = (W2 / math.sqrt(W2.shape[0])).astype(np.float32)
    w3s = (W3 / math.sqrt(W3.shape[0])).astype(np.float32)
    colscale = np.full(4 * C32, 1.0 / math.sqrt(20.0), np.float32)
    colscale[C32:2 * C32] /= math.sqrt(3.0)
    w3s = w3s * colscale[None, :]

    w1blk = np.zeros((64, 128), np.float32)
    w1blk[0:32, 0:64] = w1s
    w1blk[32:64, 64:128] = w1s
    w2blk = np.zeros((128, 128), np.float32)
    w2blk[0:64, 0:64] = w2s
    w2blk[64:128, 64:128] = w2s
    w3x1 = np.zeros((64, 256), np.float32)
    w3x1[:, 0:64] = w3s[:, 0:64]
    w3x1[:, 64:160] = np.repeat(w3s[:, 64:96], 3, axis=1)
    w3x1[:, 160:256] = np.repeat(w3s[:, 96:128], 3, axis=1)
    w3x = np.concatenate([w3x1, w3x1], axis=0)      # both partition halves
    iota = np.tile(np.arange(S, dtype=np.float32), (P, 1))

    key = (ef.shape[0], K_wt.tobytes(), len(batches))
    if key not in _CACHE:
        _CACHE[key] = _build_program(K_wt, batches, NCORES)
    nc = _CACHE[key]

    in_maps = []
    for c in range(NCORES):
        eids, loc_km, ids_km, rel_km = per_core[c]
        tab16, idx16, attrs, relpm, st3p, sT = _pack_core(
            ef, ea, eids, loc_km, ids_km, rel_km, K_wt, batches)
        in_maps.append({
            "ef": tab16,
            "idx": idx16,
            "rel": relpm,
            "attrs": attrs,
            "st3": st3p,
            "sT": sT,
            "iota": iota.astype(BF16),
            "w1": w1blk.astype(BF16),
            "w2": w2blk.astype(BF16),
            "w3": w3x.astype(BF16),
        })

    _LAST_RUN[0], _LAST_RUN[1] = nc, in_maps
    res = run_bass_kernel_spmd(nc, in_maps, core_ids=list(range(NCORES)))
    if res.exec_time_ns is not None:
        print(f"HW exec time: {res.exec_time_ns} ns")

    out = np.empty((n, 2 * P), np.float32)
    for c in range(NCORES):
        fm = res.results[c]["out"]            # [nw*S, 256]
        out[c * npc:(c + 1) * npc] = fm[:npc]
    return out


# revision 28
# speedup vs baseline: 1.0014x; 1.0014x over previous
"""Trainium2 Bass kernel for e3nn-style GNN message passing + segment-sum.

Strategy (v2 — batched bf16 pipeline, dma_gather):
  - Sharding: core c owns nodes [c*N/8, (c+1)*N/8) and receives exactly the
    edges targeting them (edge shard = mask-compaction of edge_feats rows in
    natural edge order; bf16).  No collectives: output rows are exclusive.
  - Host (numpy, index prep): windows of S=128 nodes; edges of a window are
    packed into 128-edge chunks.  Each core's edge table is split into two
    <=32768-row halves so dma_gather's int16 indices reach every row; chunks
    are half-homogeneous.  Host pre-packs per slot: gather index (wrapped-16
    int16 layout), edge attrs (s0,s0,v0x,v0y,v0z,0,0,0 — s0 duplicated so
    broadcast APs keep the DVE 2x perf mode), rel-receiver (f32, for the
    one-hot), the MLP scalars pre-transposed ([64, 512] per 8-chunk batch for
    block-diagonal matmuls), and the scalars tripled (s[c] at cols 3c+d).
  - Device (Bass/Tile, SPMD on 8 cores): per window
      * one dma_gather per table-half pulls the window's edge rows (256B/row)
      * MLP 8 chunks at a time: block-diag W1/W2 keep 128 partitions busy;
        W3 pre-expanded to 256 cols (vector gates replicated x3), one ACT
        copy yields all gates edge-major bf16
      * tensor-product messages on DVE, batched, bf16 (2x mode layouts)
      * per chunk: one-hot(rel) via tensor_scalar is_equal, then ONE
        accumulating matmul win[128 nodes, 256 feats] += oh^T @ msg
      * window flush: ACT copy PSUM->SBUF, DMA out in [node, feat] layout.
"""

import math
import os

import numpy as np
import ml_dtypes

BF16 = ml_dtypes.bfloat16

P = 128          # edges per chunk == SBUF partitions
S = 128          # nodes per window (one-hot width, PSUM partition dim)
MB = 8           # chunks per MLP/DVE batch
TS = 32768       # rows per gather table half (int16 index range)
C32 = 32         # irrep multiplicity
NCORES = 8

_CACHE = {}
_LAST_RUN = [None, None]


def last_run():
    """(nc, in_maps) of the most recent kernel() call — for external timing."""
    return _LAST_RUN[0], _LAST_RUN[1]


# ----------------------------------------------------------------- host prep

def _build_schedule(rc, n, ncores):
    """Per-core slot tables.

    Returns (K_wt [nw,2] shared chunk counts, per_core) with per_core[c] =
    (eids [cnt] global edge ids of the core's table rows (natural order),
     loc_km [C,P] int64 local table row per slot (dummy 0 on pads),
     ids_km [C,P] int64 global edge id per slot (0 on pads),
     rel_km [C,P] int64 rel receiver (-1 on pads)).
    """
    npc = n // ncores
    assert n % ncores == 0
    nw = math.ceil(npc / S)

    cores = []
    for c in range(ncores):
        eids = np.flatnonzero(rc // npc == c)          # ascending edge ids
        cnt = eids.shape[0]
        assert cnt <= 2 * TS, f"core {c} edge count {cnt} exceeds 2*{TS}"
        h = min(cnt, TS)
        loc = np.arange(cnt, dtype=np.int64)
        half = (loc >= h).astype(np.int64)
        w = (rc[eids] - c * npc) // S
        cores.append((eids, loc, half, w, h))

    K_wt = np.zeros((nw, 2), dtype=np.int64)
    for c in range(ncores):
        _, _, half, w, _ = cores[c]
        for t in (0, 1):
            cnts = np.bincount(w[half == t], minlength=nw)
            K_wt[:, t] = np.maximum(K_wt[:, t], -(-cnts // P))
    # every window needs >= 1 chunk so the PSUM accumulation group exists
    empty = (K_wt.sum(axis=1) == 0)
    K_wt[empty, 0] = 1

    C = int(K_wt.sum())
    per_core = []
    for c in range(ncores):
        eids, loc, half, w, h = cores[c]
        loc_km = np.zeros((C, P), np.int64)
        ids_km = np.zeros((C, P), np.int64)
        rel_km = np.full((C, P), -1, np.int64)
        cnts = []
        c0 = 0
        for wi in range(nw):
            for t in (0, 1):
                K = int(K_wt[wi, t])
                if K == 0:
                    continue
                sel = np.flatnonzero((w == wi) & (half == t))
                nreal = sel.shape[0]
                tot = K * P
                l = np.zeros(tot, np.int64)
                g = np.zeros(tot, np.int64)
                r = np.full(tot, -1, np.int64)
                l[:nreal] = loc[sel] - t * h
                g[:nreal] = eids[sel]
                r[:nreal] = rc[eids[sel]] - (c * npc + wi * S)
                loc_km[c0:c0 + K] = l.reshape(K, P)
                ids_km[c0:c0 + K] = g.reshape(K, P)
                rel_km[c0:c0 + K] = r.reshape(K, P)
                cnts.append(max(nreal, 1))
                c0 += K
        per_core.append((eids, loc_km, ids_km, rel_km,
                         np.asarray(cnts, np.int32)[None, :]))
    return K_wt, per_core


def _batches(K_wt):
    """Static batch list: (window, chunk_base_in_window, n_chunks)."""
    nw = K_wt.shape[0]
    out = []
    base = 0
    for w in range(nw):
        K = int(K_wt[w].sum())
        for b0 in range(0, K, MB):
            out.append((w, base, b0, min(MB, K - b0)))
        base += K
    return out


def _pack_idx16(loc_km, K_wt):
    """Wrapped-16 int16 index stream: per (w,t) group, idx i of the group at
    [i%16, i//16], replicated over the 8 partition groups -> [128, 8*C]."""
    C = loc_km.shape[0]
    flat = loc_km.reshape(C * P).astype(np.int16)        # group-major already
    wrapped = flat.reshape(C * 8, 16).T                  # [16, 8*C]
    return np.ascontiguousarray(np.tile(wrapped, (8, 1)))


def _pack_core(ef, ea, eids, loc_km, ids_km, rel_km, K_wt, batches):
    Cn = ids_km.shape[0]
    valid = rel_km >= 0

    tab = np.zeros((2 * TS, ef.shape[1]), np.float32)
    tab[:eids.shape[0]] = ef[eids]
    tab16 = tab.astype(BF16)

    idx16 = _pack_idx16(loc_km, K_wt)

    at = np.zeros((Cn, P, 8), np.float32)
    ea4 = ea[ids_km]
    at[:, :, 0] = ea4[:, :, 0]
    at[:, :, 1] = ea4[:, :, 0]
    at[:, :, 2:5] = ea4[:, :, 1:4]
    at[~valid] = 0.0
    attrs = np.ascontiguousarray(
        at.transpose(1, 0, 2).reshape(P, Cn * 8)).astype(BF16)

    relpm = np.ascontiguousarray(rel_km.T.astype(np.float32))

    s = ef[ids_km, :C32]                                  # [C,P,32]
    s[~valid] = 0.0
    st3 = np.repeat(s, 3, axis=2)
    st3p = np.ascontiguousarray(
        st3.transpose(1, 0, 2).reshape(P, Cn * 96)).astype(BF16)

    nbt = len(batches)
    sb8 = np.zeros((nbt, 8, P, C32), np.float32)
    for ib, (w, base, b0, nch) in enumerate(batches):
        g0 = base + b0
        sb8[ib, :nch] = s[g0:g0 + nch]
    sre = sb8.reshape(nbt, 2, 4, P, C32).transpose(0, 1, 4, 2, 3)
    sT = np.ascontiguousarray(
        sre.reshape(nbt, 64, 4 * P).transpose(1, 0, 2).reshape(64, nbt * 4 * P)
    ).astype(BF16)
    return tab16, idx16, attrs, relpm, st3p, sT


# ------------------------------------------------------------- device build

def _build_program(K_wt, batches, ncores):
    import concourse.bass as bass
    import concourse.bacc as bacc
    import concourse.mybir as mybir
    import concourse.tile as tile
    from concourse import library_config

    F32 = mybir.dt.float32
    BF = mybir.dt.bfloat16
    I16 = mybir.dt.int16
    AF = mybir.ActivationFunctionType
    ALU = mybir.AluOpType
    nw = K_wt.shape[0]
    Cn = int(K_wt.sum())
    nbt = len(batches)
    kstage = int(os.environ.get("KSTAGE", "6"))

    nc = bacc.Bacc("TRN2", target_bir_lowering=False, debug=False,
                   num_devices=ncores)

    ef_d = nc.dram_tensor("ef", [2 * TS, P], BF, kind="ExternalInput")
    idx_d = nc.dram_tensor("idx", [P, 8 * Cn], I16, kind="ExternalInput")
    rel_d = nc.dram_tensor("rel", [P, Cn], F32, kind="ExternalInput")
    ngroups = int((K_wt > 0).sum())
    cnt_d = nc.dram_tensor("cnt", [1, ngroups], mybir.dt.int32,
                           kind="ExternalInput")
    attrs_d = nc.dram_tensor("attrs", [P, 8 * Cn], BF, kind="ExternalInput")
    st3_d = nc.dram_tensor("st3", [P, 96 * Cn], BF, kind="ExternalInput")
    sT_d = nc.dram_tensor("sT", [64, 4 * P * nbt], BF, kind="ExternalInput")
    iota_d = nc.dram_tensor("iota", [P, S], BF, kind="ExternalInput")
    w1_d = nc.dram_tensor("w1", [64, 128], BF, kind="ExternalInput")
    w2_d = nc.dram_tensor("w2", [128, 128], BF, kind="ExternalInput")
    w3_d = nc.dram_tensor("w3", [128, 256], BF, kind="ExternalInput")
    out_d = nc.dram_tensor("out", [nw * S, 2 * P], F32, kind="ExternalOutput")

    with tile.TileContext(nc) as tc:
        with tc.tile_pool(name="const", bufs=1) as cpool, \
             tc.tile_pool(name="gef", bufs=2) as gef, \
             tc.tile_pool(name="sbat", bufs=3) as sbat, \
             tc.tile_pool(name="smsg", bufs=2) as smsg, \
             tc.tile_pool(name="soh", bufs=16) as soh, \
             tc.tile_pool(name="wfp", bufs=2) as wfp, \
             tc.tile_pool(name="pmm", bufs=1, space="PSUM") as pmm, \
             tc.tile_pool(name="pwin", bufs=2, space="PSUM") as pwin:

            nc.gpsimd.load_library(library_config.mlp)
            iota = cpool.tile([P, S], BF)
            w1b = cpool.tile([64, 128], BF)
            w2b = cpool.tile([128, 128], BF)
            w3b = cpool.tile([128, 256], BF)
            idx_sb = cpool.tile([P, 8 * Cn], I16)
            rel_sb = cpool.tile([P, Cn], F32)
            nc.sync.dma_start(iota[:], iota_d[:])
            nc.sync.dma_start(w1b[:], w1_d[:])
            nc.sync.dma_start(w2b[:], w2_d[:])
            nc.sync.dma_start(w3b[:], w3_d[:])
            nc.sync.dma_start(idx_sb[:], idx_d[:])
            nc.sync.dma_start(rel_sb[:], rel_d[:])

            # per-window bookkeeping
            efw_tiles = {}
            win_tiles = {}
            gi = 0

            for ib, (w, base, b0, nch) in enumerate(batches):
                if b0 == 0:
                    # window start: gather both table halves
                    Kw = int(K_wt[w].sum())
                    Kmax = int(K_wt.sum(axis=1).max())
                    efw = gef.tile([P, Kmax * P], BF, tag="efw", name="efw")
                    efw_tiles[w] = efw
                    coff = 0
                    for t in (0, 1):
                        K = int(K_wt[w, t])
                        if K == 0:
                            continue
                        ni = K * P
                        g0 = base + coff
                        nc.gpsimd.dma_gather(
                            efw[:, coff * P:(coff + K) * P].rearrange(
                                "p (j f) -> p j f", f=P),
                            ef_d[t * TS:(t + 1) * TS, :],
                            idx_sb[:, 8 * g0:8 * (g0 + K)],
                            ni, ni, P, single_packet=False)
                        coff += K
                    win_tiles[w] = pwin.tile([S, 2 * P], F32, tag="win",
                                             space="PSUM", name="win")
                efw = efw_tiles[w]
                win = win_tiles[w]
                gch = base + b0            # global chunk id of batch start
                Kw = int(K_wt[w].sum())

                if kstage < 2:
                    continue
                sTb = sbat.tile([64, 4 * P], BF, tag="sT")
                st3b = sbat.tile([P, MB * 96], BF, tag="st3")
                atb = sbat.tile([P, MB * 8], BF, tag="at")
                nc.sync.dma_start(sTb[:], sT_d[:, ib * 4 * P:(ib + 1) * 4 * P])
                nc.sync.dma_start(
                    st3b[:, 0:nch * 96],
                    st3_d[:, gch * 96:(gch + nch) * 96])
                nc.sync.dma_start(
                    atb[:, 0:nch * 8], attrs_d[:, gch * 8:(gch + nch) * 8])

                if kstage < 3:
                    continue
                # ---- MLP: h1/h2 partition-stacked via block-diag weights
                h1 = pmm.tile([P, 4 * P], F32, tag="h1", space="PSUM")
                nc.tensor.matmul(h1[:], w1b[:], sTb[:], start=True, stop=True)
                h1s = smsg.tile([P, 4 * P], BF, tag="h1s")
                nc.scalar.activation(h1s[:], h1[:], AF.Silu)
                h2 = pmm.tile([P, 4 * P], F32, tag="h2", space="PSUM")
                nc.tensor.matmul(h2[:], w2b[:], h1s[:], start=True, stop=True)
                h2s = smsg.tile([P, 4 * P], BF, tag="h2s")
                nc.scalar.activation(h2s[:], h2[:], AF.Silu)
                mixx = pmm.tile([P, MB * 256], F32, tag="mixx", space="PSUM")
                for k in range(nch):
                    half = (k // 4) * 64
                    lhsT = h2s[half:half + 64, (k % 4) * P:(k % 4) * P + P]
                    nc.tensor.matmul(mixx[:, k * 256:(k + 1) * 256],
                                     lhsT, w3b[half:half + 64, :],
                                     start=True, stop=True)
                mixxs = smsg.tile([P, MB * 256], BF, tag="mixxs")
                nc.scalar.activation(mixxs[:, 0:nch * 256],
                                     mixx[:, 0:nch * 256], AF.Copy)

                if kstage < 4:
                    continue
                # ---- messages (DVE, batched over nch chunks, bf16)
                efb = efw[:, b0 * P:(b0 + nch) * P]
                ef3 = efb.rearrange("p (j f) -> p j f", f=P)
                at3 = atb[:, 0:nch * 8].rearrange("p (j a) -> p j a", a=8)
                mx3 = mixxs[:, 0:nch * 256].rearrange("p (j f) -> p j f",
                                                      f=256)
                v4 = ef3[:, :, C32:4 * C32].rearrange(
                    "p j (c d) -> p j c d", d=3)
                v0b = at3[:, :, 2:5].rearrange(
                    "p j (x d) -> p j x d", x=1).to_broadcast(
                        [P, nch, C32, 3])
                s0p = at3[:, :, 0:2].rearrange("p j (x d) -> p j x d", x=1)

                t1 = smsg.tile([P, MB * 96], BF, tag="t1")
                t14 = t1[:, 0:nch * 96].rearrange(
                    "p (j c d) -> p j c d", c=C32, d=3)
                nc.vector.tensor_tensor(t14, v4, v0b, op=ALU.mult)

                ab = smsg.tile([P, MB * 64], BF, tag="ab")
                ab3 = ab[:, 0:nch * 64].rearrange("p (j f) -> p j f", f=64)
                with nc.allow_low_precision("bf16 edge dot accum"):
                    nc.vector.tensor_reduce(
                        ab3[:, :, C32:2 * C32], t14,
                        axis=mybir.AxisListType.X, op=ALU.add)
                a4 = ab3[:, :, 0:C32].rearrange("p j (c d) -> p j c d", d=2)
                sin4 = ef3[:, :, 0:C32].rearrange("p j (c d) -> p j c d", d=2)
                nc.vector.tensor_tensor(
                    a4, sin4, s0p.to_broadcast([P, nch, 16, 2]), op=ALU.mult)

                msg = smsg.tile([P, MB * 256], BF, tag="msg")
                m3 = msg[:, 0:nch * 256].rearrange("p (j f) -> p j f", f=256)
                nc.vector.tensor_tensor(m3[:, :, 0:64], ab3[:],
                                        mx3[:, :, 0:64], op=ALU.mult)

                st34 = st3b[:, 0:nch * 96].rearrange(
                    "p (j c d) -> p j c d", c=C32, d=3)
                tvv = smsg.tile([P, MB * 192], BF, tag="tvv")
                tvv3 = tvv[:, 0:nch * 192].rearrange("p (j f) -> p j f",
                                                     f=192)
                nc.vector.tensor_tensor(
                    tvv3[:, :, 0:96].rearrange("p j (c d) -> p j c d",
                                               c=C32, d=3),
                    st34, v0b, op=ALU.mult)
                vin4 = ef3[:, :, C32:4 * C32].rearrange(
                    "p j (c d) -> p j c d", d=2)
                nc.vector.tensor_tensor(
                    tvv3[:, :, 96:192].rearrange("p j (c d) -> p j c d",
                                                 c=48, d=2),
                    vin4, s0p.to_broadcast([P, nch, 48, 2]), op=ALU.mult)
                nc.vector.tensor_tensor(m3[:, :, 64:256], tvv3,
                                        mx3[:, :, 64:256], op=ALU.mult)

                if kstage < 5:
                    continue
                # ---- one-hot (one batched op) + window accumulation
                oh = soh.tile([P, MB * S], BF, tag="oh")
                nc.vector.tensor_tensor(
                    oh[:, 0:nch * S].rearrange("p (j f) -> p j f", f=S),
                    iota[:].rearrange("p (x f) -> p x f", x=1)
                           .to_broadcast([P, nch, S]),
                    rel_sb[:, gch:gch + nch].rearrange(
                        "p (j x) -> p j x", x=1).to_broadcast([P, nch, S]),
                    op=ALU.is_equal)
                for k in range(nch):
                    nc.tensor.matmul(win[:], oh[:, k * S:(k + 1) * S],
                                     msg[:, k * 256:(k + 1) * 256],
                                     start=(b0 + k == 0),
                                     stop=(b0 + k == Kw - 1))
                if b0 + nch == Kw:
                    wf = wfp.tile([S, 2 * P], F32, tag="wf")
                    nc.scalar.activation(wf[:], win[:], AF.Copy)
                    nc.sync.dma_start(out_d[w * S:(w + 1) * S, :], wf[:])
                    del win_tiles[w]

    nc.compile()
    return nc


# ------------------------------------------------------------------- kernel

def kernel(edge_feats, edge_attrs, receivers, n_nodes, W1, W2, W3):
    from concourse.bass_utils import run_bass_kernel_spmd

    ef = np.asarray(edge_feats, dtype=np.float32)
    ea = np.asarray(edge_attrs, dtype=np.float32)
    rc = np.asarray(receivers).astype(np.int64)
    n = int(n_nodes)
    W1 = np.asarray(W1, dtype=np.float32)
    W2 = np.asarray(W2, dtype=np.float32)
    W3 = np.asarray(W3, dtype=np.float32)
    npc = n // NCORES
    nw = math.ceil(npc / S)

    K_wt, per_core = _build_schedule(rc, n, NCORES)
    batches = _batches(K_wt)

    # prescaled weights; fold 1/sqrt(3) (cols 32:64) and 1/sqrt(20) into W3
    w1s = (W1 / math.sqrt(W1.shape[0])).astype(np.float32)
    w2s = (W2 / math.sqrt(W2.shape[0])).astype(np.float32)
    w3s = (W3 / math.sqrt(W3.shape[0])).astype(np.float32)
    colscale = np.full(4 * C32, 1.0 / math.sqrt(20.0), np.float32)
    colscale[C32:2 * C32] /= math.sqrt(3.0)
    w3s = w3s * colscale[None, :]

    w1blk = np.zeros((64, 128), np.float32)
    w1blk[0:32, 0:64] = w1s
    w1blk[32:64, 64:128] = w1s
    w2blk = np.zeros((128, 128), np.float32)
    w2blk[0:64, 0:64] = w2s
    w2blk[64:128, 64:128] = w2s
    w3x1 = np.zeros((64, 256), np.float32)
    w3x1[:, 0:64] = w3s[:, 0:64]
    w3x1[:, 64:160] = np.repeat(w3s[:, 64:96], 3, axis=1)
    w3x1[:, 160:256] = np.repeat(w3s[:, 96:128], 3, axis=1)
    w3x = np.concatenate([w3x1, w3x1], axis=0)      # both partition halves
    iota = np.tile(np.arange(S, dtype=np.float32), (P, 1))

    key = (ef.shape[0], K_wt.tobytes(), len(batches))
    if key not in _CACHE:
        _CACHE[key] = _build_program(K_wt, batches, NCORES)
    nc = _CACHE[key]

    in_maps = []
    for c in range(NCORES):
        eids, loc_km, ids_km, rel_km, cnts = per_core[c]
        tab16, idx16, attrs, relpm, st3p, sT = _pack_core(
            ef, ea, eids, loc_km, ids_km, rel_km, K_wt, batches)
        in_maps.append({
            "cnt": cnts,
            "ef": tab16,
            "idx": idx16,
            "rel": relpm,
            "attrs": attrs,
            "st3": st3p,
            "sT": sT,
            "iota": iota.astype(BF16),
            "w1": w1blk.astype(BF16),
            "w2": w2blk.astype(BF16),
            "w3": w3x.astype(BF16),
        })

    _LAST_RUN[0], _LAST_RUN[1] = nc, in_maps
    res = run_bass_kernel_spmd(nc, in_maps, core_ids=list(range(NCORES)))
    if res.exec_time_ns is not None:
        print(f"HW exec time: {res.exec_time_ns} ns")

    out = np.empty((n, 2 * P), np.float32)
    for c in range(NCORES):
        fm = res.results[c]["out"]            # [nw*S, 256]
        out[c * npc:(c + 1) * npc] = fm[:npc]
    return out


# revision 31
# speedup vs baseline: 1.0015x; 1.0001x over previous
"""Trainium2 Bass kernel for e3nn-style GNN message passing + segment-sum.

Strategy (v2 — batched bf16 pipeline, dma_gather):
  - Sharding: core c owns nodes [c*N/8, (c+1)*N/8) and receives exactly the
    edges targeting them (edge shard = mask-compaction of edge_feats rows in
    natural edge order; bf16).  No collectives: output rows are exclusive.
  - Host (numpy, index prep): windows of S=128 nodes; edges of a window are
    packed into 128-edge chunks.  Each core's edge table is split into two
    <=32768-row halves so dma_gather's int16 indices reach every row; chunks
    are half-homogeneous.  Host pre-packs per slot: gather index (wrapped-16
    int16 layout), edge attrs (s0,s0,v0x,v0y,v0z,0,0,0 — s0 duplicated so
    broadcast APs keep the DVE 2x perf mode), rel-receiver (f32, for the
    one-hot), the MLP scalars pre-transposed ([64, 512] per 8-chunk batch for
    block-diagonal matmuls), and the scalars tripled (s[c] at cols 3c+d).
  - Device (Bass/Tile, SPMD on 8 cores): per window
      * one dma_gather per table-half pulls the window's edge rows (256B/row)
      * MLP 8 chunks at a time: block-diag W1/W2 keep 128 partitions busy;
        W3 pre-expanded to 256 cols (vector gates replicated x3), one ACT
        copy yields all gates edge-major bf16
      * tensor-product messages on DVE, batched, bf16 (2x mode layouts)
      * per chunk: one-hot(rel) via tensor_scalar is_equal, then ONE
        accumulating matmul win[128 nodes, 256 feats] += oh^T @ msg
      * window flush: ACT copy PSUM->SBUF, DMA out in [node, feat] layout.
"""

import math
import os

import numpy as np
import ml_dtypes

BF16 = ml_dtypes.bfloat16

P = 128          # edges per chunk == SBUF partitions
S = 128          # nodes per window (one-hot width, PSUM partition dim)
MB = 8           # chunks per MLP/DVE batch
TS = 32768       # rows per gather table half (int16 index range)
C32 = 32         # irrep multiplicity
NCORES = 8

_CACHE = {}
_LAST_RUN = [None, None]


def last_run():
    """(nc, in_maps) of the most recent kernel() call — for external timing."""
    return _LAST_RUN[0], _LAST_RUN[1]


# ----------------------------------------------------------------- host prep

def _build_schedule(rc, n, ncores):
    """Per-core slot tables.

    Returns (K_wt [nw,2] shared chunk counts, per_core) with per_core[c] =
    (eids [cnt] global edge ids of the core's table rows (natural order),
     loc_km [C,P] int64 local table row per slot (dummy 0 on pads),
     ids_km [C,P] int64 global edge id per slot (0 on pads),
     rel_km [C,P] int64 rel receiver (-1 on pads)).
    """
    npc = n // ncores
    assert n % ncores == 0
    nw = math.ceil(npc / S)

    cores = []
    for c in range(ncores):
        eids = np.flatnonzero(rc // npc == c)          # ascending edge ids
        cnt = eids.shape[0]
        assert cnt <= 2 * TS, f"core {c} edge count {cnt} exceeds 2*{TS}"
        h = min(cnt, TS)
        loc = np.arange(cnt, dtype=np.int64)
        half = (loc >= h).astype(np.int64)
        w = (rc[eids] - c * npc) // S
        cores.append((eids, loc, half, w, h))

    K_wt = np.zeros((nw, 2), dtype=np.int64)
    for c in range(ncores):
        _, _, half, w, _ = cores[c]
        for t in (0, 1):
            cnts = np.bincount(w[half == t], minlength=nw)
            K_wt[:, t] = np.maximum(K_wt[:, t], -(-cnts // P))
    # every window needs >= 1 chunk so the PSUM accumulation group exists
    empty = (K_wt.sum(axis=1) == 0)
    K_wt[empty, 0] = 1

    C = int(K_wt.sum())
    per_core = []
    for c in range(ncores):
        eids, loc, half, w, h = cores[c]
        loc_km = np.zeros((C, P), np.int64)
        ids_km = np.zeros((C, P), np.int64)
        rel_km = np.full((C, P), -1, np.int64)
        cnts = []
        c0 = 0
        for wi in _worder(K_wt):
            for t in (0, 1):
                K = int(K_wt[wi, t])
                if K == 0:
                    continue
                sel = np.flatnonzero((w == wi) & (half == t))
                nreal = sel.shape[0]
                tot = K * P
                l = np.zeros(tot, np.int64)
                g = np.zeros(tot, np.int64)
                r = np.full(tot, -1, np.int64)
                l[:nreal] = loc[sel] - t * h
                g[:nreal] = eids[sel]
                r[:nreal] = rc[eids[sel]] - (c * npc + wi * S)
                loc_km[c0:c0 + K] = l.reshape(K, P)
                ids_km[c0:c0 + K] = g.reshape(K, P)
                rel_km[c0:c0 + K] = r.reshape(K, P)
                cnts.append(max(nreal, 1))
                c0 += K
        per_core.append((eids, loc_km, ids_km, rel_km,
                         np.asarray(cnts, np.int32)[None, :]))
    return K_wt, per_core


def _worder(K_wt):
    """Window processing order: descending chunk count (smallest last)."""
    nw = K_wt.shape[0]
    return sorted(range(nw), key=lambda w: -int(K_wt[w].sum()))


def _batches(K_wt):
    """Static batch list: (window, chunk_base_in_stream, b0, n_chunks).
    Stream order follows _worder."""
    out = []
    base = 0
    for w in _worder(K_wt):
        K = int(K_wt[w].sum())
        for b0 in range(0, K, MB):
            out.append((w, base, b0, min(MB, K - b0)))
        base += K
    return out


def _pack_idx16(loc_km, K_wt):
    """Wrapped-16 int16 index stream: per (w,t) group, idx i of the group at
    [i%16, i//16], replicated over the 8 partition groups -> [128, 8*C]."""
    C = loc_km.shape[0]
    flat = loc_km.reshape(C * P).astype(np.int16)        # group-major already
    wrapped = flat.reshape(C * 8, 16).T                  # [16, 8*C]
    return np.ascontiguousarray(np.tile(wrapped, (8, 1)))


def _pack_core(ef, ea, eids, loc_km, ids_km, rel_km, K_wt, batches):
    Cn = ids_km.shape[0]
    valid = rel_km >= 0

    tab = np.zeros((2 * TS, ef.shape[1]), np.float32)
    tab[:eids.shape[0]] = ef[eids]
    tab16 = tab.astype(BF16)

    idx16 = _pack_idx16(loc_km, K_wt)

    at = np.zeros((Cn, P, 8), np.float32)
    ea4 = ea[ids_km]
    at[:, :, 0] = ea4[:, :, 0]
    at[:, :, 1] = ea4[:, :, 0]
    at[:, :, 2:5] = ea4[:, :, 1:4]
    at[~valid] = 0.0
    attrs = np.ascontiguousarray(
        at.transpose(1, 0, 2).reshape(P, Cn * 8)).astype(BF16)

    relpm = np.ascontiguousarray(rel_km.T.astype(np.float32))

    s = ef[ids_km, :C32]                                  # [C,P,32]
    s[~valid] = 0.0
    st3 = np.repeat(s, 3, axis=2)
    st3p = np.ascontiguousarray(
        st3.transpose(1, 0, 2).reshape(P, Cn * 96)).astype(BF16)

    nbt = len(batches)
    sb8 = np.zeros((nbt, 8, P, C32), np.float32)
    for ib, (w, base, b0, nch) in enumerate(batches):
        g0 = base + b0
        sb8[ib, :nch] = s[g0:g0 + nch]
    sre = sb8.reshape(nbt, 2, 4, P, C32).transpose(0, 1, 4, 2, 3)
    sT = np.ascontiguousarray(
        sre.reshape(nbt, 64, 4 * P).transpose(1, 0, 2).reshape(64, nbt * 4 * P)
    ).astype(BF16)
    return tab16, idx16, attrs, relpm, st3p, sT


# ------------------------------------------------------------- device build

def _build_program(K_wt, batches, ncores):
    import concourse.bass as bass
    import concourse.bacc as bacc
    import concourse.mybir as mybir
    import concourse.tile as tile
    from concourse import library_config

    F32 = mybir.dt.float32
    BF = mybir.dt.bfloat16
    I16 = mybir.dt.int16
    AF = mybir.ActivationFunctionType
    ALU = mybir.AluOpType
    nw = K_wt.shape[0]
    Cn = int(K_wt.sum())
    nbt = len(batches)
    kstage = int(os.environ.get("KSTAGE", "6"))

    nc = bacc.Bacc("TRN2", target_bir_lowering=False, debug=False,
                   num_devices=ncores)

    ef_d = nc.dram_tensor("ef", [2 * TS, P], BF, kind="ExternalInput")
    idx_d = nc.dram_tensor("idx", [P, 8 * Cn], I16, kind="ExternalInput")
    rel_d = nc.dram_tensor("rel", [P, Cn], F32, kind="ExternalInput")
    ngroups = int((K_wt > 0).sum())
    cnt_d = nc.dram_tensor("cnt", [1, ngroups], mybir.dt.int32,
                           kind="ExternalInput")
    attrs_d = nc.dram_tensor("attrs", [P, 8 * Cn], BF, kind="ExternalInput")
    st3_d = nc.dram_tensor("st3", [P, 96 * Cn], BF, kind="ExternalInput")
    sT_d = nc.dram_tensor("sT", [64, 4 * P * nbt], BF, kind="ExternalInput")
    iota_d = nc.dram_tensor("iota", [P, S], BF, kind="ExternalInput")
    w1_d = nc.dram_tensor("w1", [64, 128], BF, kind="ExternalInput")
    w2_d = nc.dram_tensor("w2", [128, 128], BF, kind="ExternalInput")
    w3_d = nc.dram_tensor("w3", [128, 256], BF, kind="ExternalInput")
    out_d = nc.dram_tensor("out", [nw * S, 2 * P], F32, kind="ExternalOutput")

    with tile.TileContext(nc) as tc:
        with tc.tile_pool(name="const", bufs=1) as cpool, \
             tc.tile_pool(name="gef", bufs=4) as gef, \
             tc.tile_pool(name="sbat", bufs=4) as sbat, \
             tc.tile_pool(name="smsg", bufs=3) as smsg, \
             tc.tile_pool(name="soh", bufs=16) as soh, \
             tc.tile_pool(name="wfp", bufs=2) as wfp, \
             tc.tile_pool(name="pmm", bufs=1, space="PSUM") as pmm, \
             tc.tile_pool(name="pwin", bufs=2, space="PSUM") as pwin:

            nc.gpsimd.load_library(library_config.mlp)
            iota = cpool.tile([P, S], BF)
            w1b = cpool.tile([64, 128], BF)
            w2b = cpool.tile([128, 128], BF)
            w3b = cpool.tile([128, 256], BF)
            idx_sb = cpool.tile([P, 8 * Cn], I16)
            rel_sb = cpool.tile([P, Cn], F32)
            nc.sync.dma_start(iota[:], iota_d[:])
            nc.sync.dma_start(w1b[:], w1_d[:])
            nc.sync.dma_start(w2b[:], w2_d[:])
            nc.sync.dma_start(w3b[:], w3_d[:])
            nc.sync.dma_start(idx_sb[:], idx_d[:])
            nc.sync.dma_start(rel_sb[:], rel_d[:])

            # per-window bookkeeping
            efw_tiles = {}
            win_tiles = {}
            gi = 0

            for ib, (w, base, b0, nch) in enumerate(batches):
                if b0 == 0:
                    # window start: gather both table halves
                    Kw = int(K_wt[w].sum())
                    Kmax = int(K_wt.sum(axis=1).max())
                    efw = gef.tile([P, Kmax * P], BF, tag="efw", name="efw")
                    efw_tiles[w] = efw
                    coff = 0
                    for t in (0, 1):
                        K = int(K_wt[w, t])
                        if K == 0:
                            continue
                        ni = K * P
                        g0 = base + coff
                        nc.gpsimd.dma_gather(
                            efw[:, coff * P:(coff + K) * P].rearrange(
                                "p (j f) -> p j f", f=P),
                            ef_d[t * TS:(t + 1) * TS, :],
                            idx_sb[:, 8 * g0:8 * (g0 + K)],
                            ni, ni, P, single_packet=False)
                        coff += K
                    win_tiles[w] = pwin.tile([S, 2 * P], F32, tag="win",
                                             space="PSUM", name="win")
                efw = efw_tiles[w]
                win = win_tiles[w]
                gch = base + b0            # global chunk id of batch start
                Kw = int(K_wt[w].sum())

                if kstage < 2:
                    continue
                sTb = sbat.tile([64, 4 * P], BF, tag="sT")
                st3b = sbat.tile([P, MB * 96], BF, tag="st3")
                atb = sbat.tile([P, MB * 8], BF, tag="at")
                nc.sync.dma_start(sTb[:], sT_d[:, ib * 4 * P:(ib + 1) * 4 * P])
                nc.sync.dma_start(
                    st3b[:, 0:nch * 96],
                    st3_d[:, gch * 96:(gch + nch) * 96])
                nc.sync.dma_start(
                    atb[:, 0:nch * 8], attrs_d[:, gch * 8:(gch + nch) * 8])

                if kstage < 3:
                    continue
                # ---- MLP: h1/h2 partition-stacked via block-diag weights
                h1 = pmm.tile([P, 4 * P], F32, tag="h1", space="PSUM")
                nc.tensor.matmul(h1[:], w1b[:], sTb[:], start=True, stop=True)
                h1s = smsg.tile([P, 4 * P], BF, tag="h1s")
                nc.scalar.activation(h1s[:], h1[:], AF.Silu)
                h2 = pmm.tile([P, 4 * P], F32, tag="h2", space="PSUM")
                nc.tensor.matmul(h2[:], w2b[:], h1s[:], start=True, stop=True)
                h2s = smsg.tile([P, 4 * P], BF, tag="h2s")
                nc.scalar.activation(h2s[:], h2[:], AF.Silu)
                mixx = pmm.tile([P, MB * 256], F32, tag="mixx", space="PSUM")
                for k in range(nch):
                    half = (k // 4) * 64
                    lhsT = h2s[half:half + 64, (k % 4) * P:(k % 4) * P + P]
                    nc.tensor.matmul(mixx[:, k * 256:(k + 1) * 256],
                                     lhsT, w3b[half:half + 64, :],
                                     start=True, stop=True)
                mixxs = smsg.tile([P, MB * 256], BF, tag="mixxs")
                nc.scalar.activation(mixxs[:, 0:nch * 256],
                                     mixx[:, 0:nch * 256], AF.Copy)

                if kstage < 4:
                    continue
                # ---- messages (DVE, batched over nch chunks, bf16)
                efb = efw[:, b0 * P:(b0 + nch) * P]
                ef3 = efb.rearrange("p (j f) -> p j f", f=P)
                at3 = atb[:, 0:nch * 8].rearrange("p (j a) -> p j a", a=8)
                mx3 = mixxs[:, 0:nch * 256].rearrange("p (j f) -> p j f",
                                                      f=256)
                v4 = ef3[:, :, C32:4 * C32].rearrange(
                    "p j (c d) -> p j c d", d=3)
                v0b = at3[:, :, 2:5].rearrange(
                    "p j (x d) -> p j x d", x=1).to_broadcast(
                        [P, nch, C32, 3])
                s0p = at3[:, :, 0:2].rearrange("p j (x d) -> p j x d", x=1)

                t1 = smsg.tile([P, MB * 96], BF, tag="t1")
                t14 = t1[:, 0:nch * 96].rearrange(
                    "p (j c d) -> p j c d", c=C32, d=3)
                nc.vector.tensor_tensor(t14, v4, v0b, op=ALU.mult)

                ab = smsg.tile([P, MB * 64], BF, tag="ab")
                ab3 = ab[:, 0:nch * 64].rearrange("p (j f) -> p j f", f=64)
                with nc.allow_low_precision("bf16 edge dot accum"):
                    nc.vector.tensor_reduce(
                        ab3[:, :, C32:2 * C32], t14,
                        axis=mybir.AxisListType.X, op=ALU.add)
                a4 = ab3[:, :, 0:C32].rearrange("p j (c d) -> p j c d", d=2)
                sin4 = ef3[:, :, 0:C32].rearrange("p j (c d) -> p j c d", d=2)
                nc.vector.tensor_tensor(
                    a4, sin4, s0p.to_broadcast([P, nch, 16, 2]), op=ALU.mult)

                msg = smsg.tile([P, MB * 256], BF, tag="msg")
                m3 = msg[:, 0:nch * 256].rearrange("p (j f) -> p j f", f=256)
                nc.vector.tensor_tensor(m3[:, :, 0:64], ab3[:],
                                        mx3[:, :, 0:64], op=ALU.mult)

                st34 = st3b[:, 0:nch * 96].rearrange(
                    "p (j c d) -> p j c d", c=C32, d=3)
                tvv = smsg.tile([P, MB * 192], BF, tag="tvv")
                tvv3 = tvv[:, 0:nch * 192].rearrange("p (j f) -> p j f",
                                                     f=192)
                nc.vector.tensor_tensor(
                    tvv3[:, :, 0:96].rearrange("p j (c d) -> p j c d",
                                               c=C32, d=3),
                    st34, v0b, op=ALU.mult)
                vin4 = ef3[:, :, C32:4 * C32].rearrange(
                    "p j (c d) -> p j c d", d=2)
                nc.vector.tensor_tensor(
                    tvv3[:, :, 96:192].rearrange("p j (c d) -> p j c d",
                                                 c=48, d=2),
                    vin4, s0p.to_broadcast([P, nch, 48, 2]), op=ALU.mult)
                nc.vector.tensor_tensor(m3[:, :, 64:256], tvv3,
                                        mx3[:, :, 64:256], op=ALU.mult)

                if kstage < 5:
                    continue
                # ---- one-hot (one batched op) + window accumulation
                oh = soh.tile([P, MB * S], BF, tag="oh")
                nc.vector.tensor_tensor(
                    oh[:, 0:nch * S].rearrange("p (j f) -> p j f", f=S),
                    iota[:].rearrange("p (x f) -> p x f", x=1)
                           .to_broadcast([P, nch, S]),
                    rel_sb[:, gch:gch + nch].rearrange(
                        "p (j x) -> p j x", x=1).to_broadcast([P, nch, S]),
                    op=ALU.is_equal)
                for k in range(nch):
                    nc.tensor.matmul(win[:], oh[:, k * S:(k + 1) * S],
                                     msg[:, k * 256:(k + 1) * 256],
                                     start=(b0 + k == 0),
                                     stop=(b0 + k == Kw - 1))
                if b0 + nch == Kw:
                    wf = wfp.tile([S, 2 * P], F32, tag="wf")
                    nc.scalar.activation(wf[:], win[:], AF.Copy)
                    nc.sync.dma_start(out_d[w * S:(w + 1) * S, :], wf[:])
                    del win_tiles[w]

    nc.compile()
    return nc


# ------------------------------------------------------------------- kernel

def kernel(edge_feats, edge_attrs, receivers, n_nodes, W1, W2, W3):
    from concourse.bass_utils import run_bass_kernel_spmd

    ef = np.asarray(edge_feats, dtype=np.float32)
    ea = np.asarray(edge_attrs, dtype=np.float32)
    rc = np.asarray(receivers).astype(np.int64)
    n = int(n_nodes)
    W1 = np.asarray(W1, dtype=np.float32)
    W2 = np.asarray(W2, dtype=np.float32)
    W3 = np.asarray(W3, dtype=np.float32)
    npc = n // NCORES
    nw = math.ceil(npc / S)

    K_wt, per_core = _build_schedule(rc, n, NCORES)
    batches = _batches(K_wt)

    # prescaled weights; fold 1/sqrt(3) (cols 32:64) and 1/sqrt(20) into W3
    w1s = (W1 / math.sqrt(W1.shape[0])).astype(np.float32)
    w2s = (W2 / math.sqrt(W2.shape[0])).astype(np.float32)
    w3s = (W3 / math.sqrt(W3.shape[0])).astype(np.float32)
    colscale = np.full(4 * C32, 1.0 / math.sqrt(20.0), np.float32)
    colscale[C32:2 * C32] /= math.sqrt(3.0)
    w3s = w3s * colscale[None, :]

    w1blk = np.zeros((64, 128), np.float32)
    w1blk[0:32, 0:64] = w1s
    w1blk[32:64, 64:128] = w1s
    w2blk = np.zeros((128, 128), np.float32)
    w2blk[0:64, 0:64] = w2s
    w2blk[64:128, 64:128] = w2s
    w3x1 = np.zeros((64, 256), np.float32)
    w3x1[:, 0:64] = w3s[:, 0:64]
    w3x1[:, 64:160] = np.repeat(w3s[:, 64:96], 3, axis=1)
    w3x1[:, 160:256] = np.repeat(w3s[:, 96:128], 3, axis=1)
    w3x = np.concatenate([w3x1, w3x1], axis=0)      # both partition halves
    iota = np.tile(np.arange(S, dtype=np.float32), (P, 1))

    key = (ef.shape[0], K_wt.tobytes(), len(batches))
    if key not in _CACHE:
        _CACHE[key] = _build_program(K_wt, batches, NCORES)
    nc = _CACHE[key]

    in_maps = []
    for c in range(NCORES):
        eids, loc_km, ids_km, rel_km, cnts = per_core[c]
        tab16, idx16, attrs, relpm, st3p, sT = _pack_core(
            ef, ea, eids, loc_km, ids_km, rel_km, K_wt, batches)
        in_maps.append({
            "cnt": cnts,
            "ef": tab16,
            "idx": idx16,
            "rel": relpm,
            "attrs": attrs,
            "st3": st3p,
            "sT": sT,
            "iota": iota.astype(BF16),
            "w1": w1blk.astype(BF16),
            "w2": w2blk.astype(BF16),
            "w3": w3x.astype(BF16),
        })

    _LAST_RUN[0], _LAST_RUN[1] = nc, in_maps
    res = run_bass_kernel_spmd(nc, in_maps, core_ids=list(range(NCORES)))
    if res.exec_time_ns is not None:
        print(f"HW exec time: {res.exec_time_ns} ns")

    out = np.empty((n, 2 * P), np.float32)
    for c in range(NCORES):
        fm = res.results[c]["out"]            # [nw*S, 256]
        out[c * npc:(c + 1) * npc] = fm[:npc]
    return out
